# revision 79
# baseline (speedup 1.0000x reference)
"""Trainium2 Bass kernel for nn_Attention_84739704750279.

Full module: out = (softmax(LN(x) Wq (LN(x) Wk)^T / sqrt(64)) (LN(x) Wv)) Wout

Sharding across 8 NeuronCores: batch (2) x head-groups (4 heads each, 4
groups). Each core computes a partial output [2048, 1024] for its batch using
only its 4 heads; the host sums the 4 partials per batch (row-parallel Wout).

Per-core program (single Bass program, SPMD over 8 cores). All PE operands
are bf16 (x is pre-converted on host; PSUM accumulation stays f32); the
harness error gate is 2e-2 and the bf16 pipeline lands ~6e-3.

Phase A -- LN + transpose + K/V projections (PE-bound, ~50us):
  - x streams in per 256 rows (bf16, halved DMA bytes); weights staggered
    between x chunks in first-use order.
  - bn_stats/bn_aggr on DVE; rstd via Act Sqrt + DVE reciprocal; the
    normalize itself on Pool (otherwise idle), writing bf16 h rows.
  - PE transposes h -> hT via identity matmuls (bf16 = 1 cycle/row), copies
    to SBUF on Act (last chunk on DVE so Act can load the exp table early).
  - V projection per row-tile right after its transposes; K projections of
    chunk cb-1 interleave into chunk cb's row-tile stream (fills DMA/LN
    latency). vaug carries 64 all-ones rows so the context matmul emits the
    softmax denominator replicated across 64 partitions.
  - Q projections and the last chunk's V are NOT done here: they are
    deferred into the phase B backlog, so attention starts ~14us earlier.

Phase B -- attention (Act/exp-bound, ~150us): per (q-block 512, head):
  - scores sT = k_tile^T q in groups of 2 k-tiles into a double-buffered
    PSUM pool; exp fused into the PSUM->SBUF copy on Act (scale=1/8 folded
    in; scores are O(+-8) so no max subtraction needed); ctxT accumulated
    over 16 k-tiles per head.
  - A credit-based backlog scheduler drains deferred PE work (lagged ctx
    groups, head normalizes, out-projection pieces, deferred Q/V) at ~1.5
    matmul-units between exp groups: the Act engine gets a new scores group
    every ~1us and is never starved behind a PE burst. Backlog items are
    tagged with their (q-block, head) pair: force_old() emits everything
    older than the previous head right before a head's tiles are allocated
    (pool ring reuse follows emission order, so a lagged read must never
    outlive its pT/psC/stg ring slot), and the performance trim drain_to()
    fires just after the head's first exp so its burst hides behind Act.
  - Normalize = DVE reciprocal of the replicated denominator rows + one
    tensor_tensor multiply (no partition broadcast).
  - Out-projection is staged per q-block and stored per 256 rows; for the
    last q-block it is split by inner-dim half (fo=0 runs during heads 2-3;
    after the last exp only the fo=1 halves remain -- paired per row-chunk
    into a borrowed scores-ring tile with a single [128, 1024] merge-add --
    and the last head's normalize is emitted per row-chunk slice so the
    first fo=1 piece starts as soon as its slice is ready).

Output partials are stored in bf16 (each core's result is 1 of 4 partial
sums; the host accumulates in f32), halving store traffic.

TimelineSim: 322.5us (baseline) -> 198.3us predicted; measured rel l2 err
6.1e-3 on hardware vs the 2e-2 harness gate.
"""

import numpy as np

NUM_HEAD = 16
HEAD_DIM = 64
DIM = 1024
INNER = NUM_HEAD * HEAD_DIM
B = 2
N = 2048

P = 128
NH = 4            # heads per core
FQK = 2 * NH * HEAD_DIM   # 512 (q block then k block)
FV = NH * HEAD_DIM        # 256
QB = 512          # q-block width for attention
NKT = N // P      # 16 k tiles
DT = DIM // P     # 8 d tiles
GK = 2            # k-tiles per exp group (PSUM double buffered)
SCALE = HEAD_DIM ** -0.5

_CACHE = {}


def _build(apply_affine: bool):
    import concourse.bass as bass
    import concourse.mybir as mybir
    import concourse.tile as tile
    from concourse import bacc
    from concourse.masks import make_identity

    f32 = mybir.dt.float32
    bf16 = mybir.dt.bfloat16
    AF = mybir.ActivationFunctionType
    MUL = mybir.AluOpType.mult

    nc = bacc.Bacc()
    x = nc.declare_dram_parameter("x", [N, DIM], bf16, isOutput=False)
    wqk = nc.declare_dram_parameter("wqk", [DIM, FQK], bf16, isOutput=False)
    wv = nc.declare_dram_parameter("wv", [DIM, FV], bf16, isOutput=False)
    wout = nc.declare_dram_parameter("wout", [FV, DIM], bf16, isOutput=False)
    if apply_affine:
        gamma = nc.declare_dram_parameter("gamma", [DIM], f32, isOutput=False)
        beta = nc.declare_dram_parameter("beta", [DIM], f32, isOutput=False)
    # Output partials in bf16: each core's result is one of 4 partial sums
    # the host accumulates in f32, so the extra rounding is ~1e-3 relative;
    # halves the store bytes and shortens the kernel tail.
    out = nc.declare_dram_parameter("out", [N, DIM], bf16, isOutput=True)
    # The last q-block's fo=1 out-projection half is stored as a separate
    # partial (host adds it): the on-device [128,1024] merge-adds were the
    # kernel's last serialized DVE work, and this lets the fo=0 half store
    # early and the fo=1 copies alternate Act/DVE after the final exp.
    out2 = nc.declare_dram_parameter("out2", [QB, DIM], bf16, isOutput=True)

    with tile.TileContext(nc) as tc:
        with (
            tc.tile_pool(name="const", bufs=1) as const,
            tc.tile_pool(name="persist", bufs=1) as persist,
        ):
            ident = const.tile([P, P], bf16)
            make_identity(nc, ident)
            eps_sb = const.tile([P, 1], f32)
            nc.vector.memset(eps_sb, 1e-5)

            # Weight tiles declared here, DMAs emitted inside phase A after
            # the first x chunk so the x pipeline fills first.
            wqk_sb = const.tile([P, DT, FQK], bf16)
            wv_sb = const.tile([P, DT, FV], bf16)
            wout_sb = const.tile([P, FV // P, DIM], bf16)
            if apply_affine:
                gamma_sb = const.tile([P, DIM], f32)
                nc.sync.dma_start(
                    out=gamma_sb,
                    in_=bass.AP(tensor=gamma.tensor, offset=gamma.offset,
                                ap=[[0, P]] + list(gamma.ap)),
                )
                beta_sb = const.tile([P, DIM], f32)
                nc.sync.dma_start(
                    out=beta_sb,
                    in_=bass.AP(tensor=beta.tensor, offset=beta.offset,
                                ap=[[0, P]] + list(beta.ap)),
                )

            # Persistent activations. vaug rows 64-127 are all-ones: the ctx
            # matmul then yields the softmax denominator replicated across
            # 64 partitions, so no partition-broadcast is needed to
            # normalize.
            qkT = persist.tile([P, FQK // P, N], bf16)      # [128, 4, 2048]
            vaug = persist.tile([P, NKT, NH, 2 * HEAD_DIM], bf16)
            ctxa = persist.tile([P, FV // P, N], bf16)      # [128, 2, 2048]
            nc.gpsimd.memset(vaug[:, :, :, HEAD_DIM:], 1.0)

            # ---------------- Phases A+B, SBUF pools shared ----------------
            with (
                tc.tile_pool(name="xln", bufs=4) as xp,
                tc.tile_pool(name="stat", bufs=10) as sp,
                tc.tile_pool(name="hrow", bufs=6) as hrp,
                tc.tile_pool(name="ht", bufs=4) as hp,
                tc.tile_pool(name="pT", bufs=2) as ppool,
                tc.tile_pool(name="small", bufs=4) as smp,
                tc.tile_pool(name="ostg", bufs=2) as osg,
            ):
                # ---- Phase A: LN, transpose, K projections, v ----
                # Q projections are deferred into the phase B backlog (the
                # q-block qb only needs its Q when its attention runs), which
                # lets the Act-bound attention phase start ~14us earlier.
                hTs = {}
                q_items = {}    # cb -> [(cost, fn), ...] run in phase B

                def make_q_items(cb):
                    items = []
                    holder = {}
                    for fo in (0, 1):
                        for k in range(DT // 2):
                            def itemfn(cb=cb, fo=fo, k=k, holder=holder):
                                if k == 0:
                                    # Shares the out-projection ring ("ps"):
                                    # FIFO order guarantees no other "ps"
                                    # alloc lands mid-accumulation.
                                    holder[fo] = psO.tile([P, QB], f32,
                                                          name="ps",
                                                          uniquify=True)
                                ps = holder[fo]
                                hT = hTs[cb]
                                for dti in (2 * k, 2 * k + 1):
                                    nc.tensor.matmul(
                                        ps,
                                        lhsT=wqk_sb[:, dti,
                                                    fo * P:(fo + 1) * P],
                                        rhs=hT[:, dti, :],
                                        start=(dti == 0),
                                        stop=(dti == DT - 1),
                                    )
                                if k == DT // 2 - 1:
                                    nc.vector.tensor_copy(
                                        out=qkT[:, fo,
                                                cb * QB:(cb + 1) * QB],
                                        in_=ps)
                            items.append((2, itemfn))
                    return items

                with (
                    tc.tile_pool(name="pstr", bufs=2, space="PSUM") as pptr,
                    tc.tile_pool(name="psqk", bufs=2, space="PSUM") as ppqk,
                ):
                    def k_proj(cb, fo, on_act=True):
                        ps = ppqk.tile([P, QB], f32, tag="qk")
                        hT = hTs[cb]
                        for dti in range(DT):
                            nc.tensor.matmul(
                                ps,
                                lhsT=wqk_sb[:, dti, fo * P:(fo + 1) * P],
                                rhs=hT[:, dti, :],
                                start=(dti == 0), stop=(dti == DT - 1),
                            )
                        dst = qkT[:, fo, cb * QB:(cb + 1) * QB]
                        if on_act:
                            nc.scalar.copy(out=dst, in_=ps)
                        else:
                            nc.vector.tensor_copy(out=dst, in_=ps)

                    def v_proj(cb, rsub, hT):
                        ps = ppqk.tile([P, FV], f32, tag="v")
                        for dti in range(DT):
                            nc.tensor.matmul(
                                ps,
                                lhsT=hT[:, dti, rsub * P:(rsub + 1) * P],
                                rhs=wv_sb[:, dti, :],
                                start=(dti == 0), stop=(dti == DT - 1),
                            )
                        nc.scalar.copy(
                            out=vaug[:, cb * 4 + rsub, :, 0:HEAD_DIM],
                            in_=ps.rearrange("p (h d) -> p h d", h=NH),
                        )

                    def make_v_items(cb):
                        # Deferred V projections for the last chunk: run at
                        # the head of the phase B backlog (first needed by
                        # ctx group 6, ~12 exp groups in).
                        items = []
                        for rsub in range(QB // P):
                            holder = {}
                            for piece in range(2):
                                def itemfn(cb=cb, rsub=rsub, piece=piece,
                                           holder=holder):
                                    if piece == 0:
                                        holder[0] = psO.tile(
                                            [P, QB], f32, name="ps",
                                            uniquify=True)
                                    ps = holder[0][:, 0:FV]
                                    hT = hTs[cb]
                                    for dti in range(4 * piece,
                                                     4 * piece + 4):
                                        nc.tensor.matmul(
                                            ps,
                                            lhsT=hT[:, dti,
                                                    rsub * P:(rsub + 1) * P],
                                            rhs=wv_sb[:, dti, :],
                                            start=(dti == 0),
                                            stop=(dti == DT - 1),
                                        )
                                    if piece == 1:
                                        nc.vector.tensor_copy(
                                            out=vaug[:, cb * 4 + rsub, :,
                                                     0:HEAD_DIM],
                                            in_=ps.rearrange(
                                                "p (h d) -> p h d", h=NH),
                                        )
                                items.append((2, itemfn))
                        return items

                    # K projections of chunk cb-1 are interleaved into chunk
                    # cb's row-tile stream to fill PE during DMA/LN latency.
                    pending_k = []
                    for cb in range(N // QB):      # 4 chunks of 512 rows
                        hT = hp.tile([P, DT, QB], bf16)
                        hTs[cb] = hT
                        last = cb == N // QB - 1
                        for half in range(2):      # x DMA per 256 rows
                            r0 = cb * QB + half * (QB // 2)
                            xt = xp.tile([P, 2, DIM], bf16)
                            if cb == 0 and half == 0:
                                # Two half-size loads so the first LN starts
                                # as early as possible.
                                for a2 in range(2):
                                    nc.sync.dma_start(
                                        out=xt[:, a2],
                                        in_=x[r0 + a2 * P:
                                              r0 + (a2 + 1) * P, :])
                            else:
                                nc.sync.dma_start(
                                    out=xt,
                                    in_=x[r0:r0 + QB // 2, :].rearrange(
                                        "(a p) d -> p a d", p=P))
                            # Weight loads staggered between x chunks, in
                            # first-use order (wv -> wqk -> wout), so no x
                            # load waits behind a weight it doesn't need yet.
                            if cb == 0 and half == 0:
                                nc.sync.dma_start(
                                    out=wv_sb,
                                    in_=wv.rearrange("(o p) f -> p o f",
                                                     p=P))
                            elif cb == 0 and half == 1:
                                # K columns (fo 2,3) first: the first K
                                # projection needs them ~1.2us before the Q
                                # half is ever read.
                                nc.sync.dma_start(
                                    out=wqk_sb[:, :, FQK // 2:],
                                    in_=wqk[:, FQK // 2:].rearrange(
                                        "(o p) f -> p o f", p=P))
                                nc.sync.dma_start(
                                    out=wqk_sb[:, :, 0:FQK // 2],
                                    in_=wqk[:, 0:FQK // 2].rearrange(
                                        "(o p) f -> p o f", p=P))
                            elif cb == 1 and half == 0:
                                nc.sync.dma_start(
                                    out=wout_sb,
                                    in_=wout.rearrange("(o p) f -> p o f",
                                                       p=P))
                            for a in range(2):
                                rsub = half * 2 + a
                                xv = xt[:, a].rearrange("p (s f) -> p s f",
                                                        s=2)
                                stats = sp.tile([P, 2, 6], f32)
                                nc.vector.bn_stats(out=stats[:, 0],
                                                   in_=xv[:, 0])
                                nc.vector.bn_stats(out=stats[:, 1],
                                                   in_=xv[:, 1])
                                mv = sp.tile([P, 2], f32)
                                nc.vector.bn_aggr(out=mv, in_=stats)
                                rstd = sp.tile([P, 1], f32)
                                nc.scalar.activation(out=rstd,
                                                     in_=mv[:, 1:2],
                                                     func=AF.Sqrt,
                                                     bias=eps_sb)
                                nc.vector.reciprocal(out=rstd, in_=rstd)
                                hrow = hrp.tile([P, DIM], bf16)
                                if apply_affine:
                                    xf = sp.tile([P, DIM], f32, tag="xf")
                                    nc.vector.tensor_scalar(
                                        out=xf, in0=xt[:, a],
                                        scalar1=mv[:, 0:1], scalar2=rstd,
                                        op0=mybir.AluOpType.subtract,
                                        op1=MUL,
                                    )
                                    nc.vector.tensor_mul(out=xf, in0=xf,
                                                         in1=gamma_sb)
                                    nc.vector.tensor_add(out=xf, in0=xf,
                                                         in1=beta_sb)
                                    nc.vector.tensor_copy(out=hrow, in_=xf)
                                elif cb == 0 and half == 0:
                                    # DVE normalize for the very first
                                    # row-tiles: shorter latency chain than
                                    # Pool while the pipeline fills.
                                    nc.vector.tensor_scalar(
                                        out=hrow, in0=xt[:, a],
                                        scalar1=mv[:, 0:1], scalar2=rstd,
                                        op0=mybir.AluOpType.subtract,
                                        op1=MUL,
                                    )
                                else:
                                    nc.gpsimd.tensor_scalar(
                                        out=hrow, in0=xt[:, a],
                                        scalar1=mv[:, 0:1], scalar2=rstd,
                                        op0=mybir.AluOpType.subtract,
                                        op1=MUL,
                                    )
                                for dhalf in range(2):
                                    tp = pptr.tile([P, 4, P], bf16)
                                    for j in range(4):
                                        dti = dhalf * 4 + j
                                        nc.tensor.transpose(
                                            tp[:, j],
                                            hrow[:, dti * P:(dti + 1) * P],
                                            ident)
                                    dst = hT[:, dhalf * 4:(dhalf + 1) * 4,
                                             rsub * P:(rsub + 1) * P]
                                    # Last chunk's copies on DVE: the Act
                                    # engine winds down early so its exp
                                    # table load overlaps phase A's tail.
                                    if last:
                                        nc.vector.tensor_copy(out=dst,
                                                              in_=tp)
                                    else:
                                        nc.scalar.copy(out=dst, in_=tp)
                                if not last:
                                    # v right away keeps PE busy during the
                                    # next row-tile's DMA/LN latency.
                                    v_proj(cb, rsub, hT)
                                if pending_k:
                                    pending_k.pop(0)()
                        if last:
                            for f in pending_k:
                                f()
                            pending_k = []
                            # K of the last chunk on DVE so the Act engine is
                            # free to switch to the exp table immediately.
                            k_proj(cb, 2, on_act=False)
                            k_proj(cb, 3, on_act=False)
                            q_items[cb] = make_q_items(cb)
                            v_items_last = make_v_items(cb)
                        else:
                            pending_k.extend(
                                (lambda fo=fo, cb=cb: k_proj(cb, fo,
                                                             on_act=False))
                                for fo in (2, 3))
                            if cb == 0:
                                # qb0 needs its Q before attention starts.
                                pending_k.extend(
                                    (lambda fo=fo: k_proj(0, fo,
                                                          on_act=False))
                                    for fo in (0, 1))
                            else:
                                q_items[cb] = make_q_items(cb)

                # ---- Phase B: attention ----
                with (
                    tc.tile_pool(name="pss", bufs=2, space="PSUM") as psS,
                    tc.tile_pool(name="psc", bufs=2, space="PSUM") as psC,
                    tc.tile_pool(name="pso", bufs=2, space="PSUM") as psO,
                ):
                    # Backlog scheduler: PE side-work (lagged ctx groups,
                    # head normalize, out-projection pieces, deferred Q
                    # projections) is queued with a PE-cost weight and
                    # drained at a bounded matmul budget between scores/exp
                    # groups, so the Act engine (the phase B bottleneck) is
                    # fed a new scores group every ~1us and never sits
                    # behind a PE burst.
                    backlog = []   # (pe_matmul_cost, fn)

                    credit = [0.0]

                    def drain(rate):
                        # Credit-carryover drain: averages `rate` matmuls of
                        # backlog per call without bursting PE between two
                        # scores groups (a burst starves the Act engine).
                        credit[0] += rate
                        while backlog and credit[0] >= backlog[0][0]:
                            cost, fn, _ = backlog.pop(0)
                            credit[0] -= cost
                            fn()
                        if not backlog:
                            credit[0] = 0.0

                    def drain_to(maxlen):
                        # Performance trim: bound the standing queue; called
                        # right after a head's first exp so the burst hides
                        # behind Act execution.
                        while len(backlog) > maxlen:
                            cost, fn, _ = backlog.pop(0)
                            fn()

                    def force_old(pair):
                        # Ring-safety: pool ring reuse is by emission order;
                        # a lagged read emitted after its ring buffer wraps
                        # (pT/psC/stg rings are 2 deep) would read clobbered
                        # data. Before pair p's tiles are allocated, every
                        # item from pair <= p-2 must have been emitted.
                        while backlog and backlog[0][2] <= pair - 2:
                            cost, fn, _ = backlog.pop(0)
                            fn()

                    def outproj_do(qb, qc4, do, stg):
                        qc = qb * (QB // P) + qc4
                        ps = psO.tile([P, QB], f32)
                        for fo in range(FV // P):  # 2
                            nc.tensor.matmul(
                                ps,
                                lhsT=ctxa[:, fo, qc * P:(qc + 1) * P],
                                rhs=wout_sb[:, fo,
                                            do * (DIM // 2):
                                            (do + 1) * (DIM // 2)],
                                start=(fo == 0), stop=(fo == FV // P - 1),
                            )
                        dst = stg[:, qc4,
                                  do * (DIM // 2):(do + 1) * (DIM // 2)]
                        nc.vector.tensor_copy(out=dst, in_=ps)
                        if qc4 % 2 == 1 and do == 1:
                            # Store per 256 rows: bounds the kernel tail.
                            r0 = qb * QB + (qc4 - 1) * P
                            nc.sync.dma_start(
                                out=out[r0:r0 + 2 * P, :].rearrange(
                                    "(a p) d -> p a d", p=P),
                                in_=stg[:, qc4 - 1:qc4 + 1, :])

                    backlog.extend((c, f, -1) for c, f in v_items_last)
                    def outproj_fo1_pair(qb, qc4, stg):
                        # Both column halves into one borrowed scores-ring
                        # tile, then a straight PSUM->SBUF copy (alternating
                        # Act/DVE -- Act is idle after the last exp) and a
                        # store to the separate out2 partial; the host adds
                        # it to the fo=0 half.  Runs only in the kernel tail
                        # (scores ring idle; allocations are consecutive in
                        # the final drain, so ring reuse stays
                        # emission-ordered).
                        qc = qb * (QB // P) + qc4
                        ps = psS.tile([P, GK * QB], f32, name="ps",
                                      uniquify=True)
                        for do in range(2):
                            nc.tensor.matmul(
                                ps[:, do * (DIM // 2):(do + 1) * (DIM // 2)],
                                lhsT=ctxa[:, 1, qc * P:(qc + 1) * P],
                                rhs=wout_sb[:, 1,
                                            do * (DIM // 2):
                                            (do + 1) * (DIM // 2)],
                                start=True, stop=True,
                            )
                        dst = stg[:, qc4, :]
                        if qc4 % 2 == 0:
                            nc.scalar.copy(out=dst, in_=ps[:, 0:DIM])
                        else:
                            nc.vector.tensor_copy(out=dst, in_=ps[:, 0:DIM])
                        nc.sync.dma_start(
                            out=out2[qc4 * P:(qc4 + 1) * P, :],
                            in_=dst)

                    def outproj_fo_half(qb, qc4, do, stg, fo):
                        qc = qb * (QB // P) + qc4
                        ps = psO.tile([P, QB], f32, name="ps",
                                      uniquify=True)
                        nc.tensor.matmul(
                            ps,
                            lhsT=ctxa[:, fo, qc * P:(qc + 1) * P],
                            rhs=wout_sb[:, fo,
                                        do * (DIM // 2):
                                        (do + 1) * (DIM // 2)],
                            start=True, stop=True,
                        )
                        dst = stg[:, qc4,
                                  do * (DIM // 2):(do + 1) * (DIM // 2)]
                        if fo == 0:
                            nc.vector.tensor_copy(out=dst, in_=ps)
                            if do == 1:
                                # fo=0 half stores early (mid phase B); the
                                # fo=1 half goes to out2 in the tail.
                                r0 = qb * QB + qc4 * P
                                nc.sync.dma_start(
                                    out=out[r0:r0 + P, :],
                                    in_=stg[:, qc4, :])
                        else:
                            nc.vector.tensor_tensor(
                                dst, dst, ps, mybir.AluOpType.add)
                            if do == 1:
                                r0 = qb * QB + qc4 * P
                                nc.sync.dma_start(
                                    out=out[r0:r0 + P, :],
                                    in_=stg[:, qc4, :])

                    for qb in range(N // QB):
                        if qb + 1 in q_items:
                            backlog.extend(
                                (c, f, qb * NH)
                                for c, f in q_items.pop(qb + 1))
                        for h in range(NH):
                            pair = qb * NH + h
                            force_old(pair)
                            p0 = (h % 2) * HEAD_DIM
                            fo_q = h // 2
                            fo_k = FQK // P // 2 + h // 2  # k in fo 2,3
                            qT_h = qkT[p0:p0 + HEAD_DIM, fo_q,
                                       qb * QB:(qb + 1) * QB]
                            kT_h = qkT[p0:p0 + HEAD_DIM, fo_k, :]
                            pT = ppool.tile([P, NKT, QB], bf16)
                            cps = psC.tile([2 * HEAD_DIM, QB], f32)

                            def ctx_group(g, pT=pT, cps=cps, h=h):
                                for j in range(GK):
                                    kt = g * GK + j
                                    nc.tensor.matmul(
                                        cps,
                                        lhsT=vaug[:, kt, h, :],
                                        rhs=pT[:, kt, :],
                                        start=(kt == 0),
                                        stop=(kt == NKT - 1),
                                    )

                            def finish_head(cps=cps, p0=p0, h=h, qb=qb,
                                            sliced=False):
                                recipb = smp.tile([HEAD_DIM, QB], f32,
                                                  tag="recipb")
                                nc.vector.reciprocal(
                                    out=recipb,
                                    in_=cps[HEAD_DIM:2 * HEAD_DIM, :])
                                n0 = qb * QB
                                if sliced:
                                    # Tail: per row-chunk slices so the
                                    # first fo=1 out-proj piece starts as
                                    # soon as its slice is normalized.
                                    for qc4 in range(QB // P):
                                        sl = slice(qc4 * P, (qc4 + 1) * P)
                                        nc.vector.tensor_tensor(
                                            ctxa[p0:p0 + HEAD_DIM, h // 2,
                                                 n0 + qc4 * P:
                                                 n0 + (qc4 + 1) * P],
                                            cps[0:HEAD_DIM, sl],
                                            recipb[:, sl], MUL)
                                else:
                                    dst = ctxa[p0:p0 + HEAD_DIM, h // 2,
                                               n0:n0 + QB]
                                    nc.vector.tensor_tensor(
                                        dst, cps[0:HEAD_DIM, :], recipb,
                                        MUL)

                            for g in range(NKT // GK):
                                ps = psS.tile([P, GK * QB], f32)
                                for j in range(GK):
                                    kt = g * GK + j
                                    nc.tensor.matmul(
                                        ps[:, j * QB:(j + 1) * QB],
                                        lhsT=kT_h[:, kt * P:(kt + 1) * P],
                                        rhs=qT_h,
                                        start=True, stop=True,
                                    )
                                nc.scalar.activation(
                                    out=pT[:, g * GK:(g + 1) * GK, :],
                                    in_=ps,
                                    func=AF.Exp, scale=float(SCALE))
                                if g == 0:
                                    # Backlog cap fires after the head's
                                    # first exp is in flight: the forced
                                    # burst overlaps Act instead of delaying
                                    # the scores Act is waiting for.
                                    drain_to(14)
                                if g >= 1:
                                    backlog.append(
                                        (GK,
                                         lambda g=g, f=ctx_group: f(g - 1),
                                         pair))
                                drain(1.5)
                            backlog.append(
                                (GK, lambda f=ctx_group: f(NKT // GK - 1),
                                 pair))
                            last_pair = (qb == N // QB - 1 and h == NH - 1)
                            backlog.append(
                                (0,
                                 (lambda f=finish_head: f(sliced=True))
                                 if last_pair else finish_head,
                                 pair))
                            if qb == N // QB - 1 and h == 1:
                                # Last q-block: the fo=0 half of the
                                # out-projection only needs heads 0-1, so it
                                # runs during heads 2-3; only the fo=1 half
                                # (+ add + store) remains after the last
                                # exp, shortening the kernel tail.
                                stg3 = osg.tile([P, QB // P, DIM], bf16,
                                                name="stg", uniquify=True)
                                backlog.extend(
                                    (1, lambda qc4=qc4, do=do:
                                     outproj_fo_half(N // QB - 1, qc4, do,
                                                     stg3, 0), pair)
                                    for qc4 in range(QB // P)
                                    for do in range(2))
                            if qb == N // QB - 1 and h == 3:
                                backlog.extend(
                                    (2, lambda qc4=qc4:
                                     outproj_fo1_pair(N // QB - 1, qc4,
                                                      stg3), pair)
                                    for qc4 in range(QB // P))
                        if qb < N // QB - 1:
                            stg = osg.tile([P, QB // P, DIM], bf16)
                            backlog.extend(
                                (2, lambda qb=qb, qc4=qc4, do=do, stg=stg:
                                 outproj_do(qb, qc4, do, stg),
                                 qb * NH + NH - 1)
                                for qc4 in range(QB // P) for do in range(2))
                    drain_to(0)
    nc.finalize()
    return nc


def _get_nc(apply_affine: bool):
    key = ("nc", apply_affine)
    if key not in _CACHE:
        _CACHE[key] = _build(apply_affine)
    return _CACHE[key]


def kernel(x, ln_gamma, ln_beta, w_qkv, w_out, _trace=False):
    from concourse.bass_utils import run_bass_kernel_spmd
    from concourse import mybir

    npbf16 = mybir.dt.np(mybir.dt.bfloat16)

    x = np.ascontiguousarray(np.asarray(x, dtype=np.float32))
    ln_gamma = np.asarray(ln_gamma, dtype=np.float32)
    ln_beta = np.asarray(ln_beta, dtype=np.float32)
    w_qkv = np.asarray(w_qkv, dtype=np.float32)
    w_out = np.asarray(w_out, dtype=np.float32)

    apply_affine = not (np.all(ln_gamma == 1.0) and np.all(ln_beta == 0.0))
    nc = _get_nc(apply_affine)

    wq = w_qkv[:, 0 * INNER:1 * INNER]
    wk = w_qkv[:, 1 * INNER:2 * INNER]
    wv = w_qkv[:, 2 * INNER:3 * INNER]

    in_maps = []
    for c in range(8):
        bi, hg = divmod(c, 4)
        fs = hg * NH * HEAD_DIM
        fe = fs + NH * HEAD_DIM
        m = {
            "x": np.ascontiguousarray(x[bi]).astype(npbf16),
            "wqk": np.ascontiguousarray(
                np.concatenate([wq[:, fs:fe], wk[:, fs:fe]],
                               axis=1)).astype(npbf16),
            "wv": np.ascontiguousarray(wv[:, fs:fe]).astype(npbf16),
            "wout": np.ascontiguousarray(w_out[fs:fe, :]).astype(npbf16),
        }
        if apply_affine:
            m["gamma"] = ln_gamma
            m["beta"] = ln_beta
        in_maps.append(m)

    import os
    try:
        res = run_bass_kernel_spmd(nc, in_maps, core_ids=list(range(8)),
                                   trace=_trace)
    except ModuleNotFoundError:
        # Tracing hooks unavailable in this environment; run untraced
        # (BASS_NEVER_TRACE also overrides a BASS_TRACE env setting).
        prev = os.environ.get("BASS_NEVER_TRACE")
        os.environ["BASS_NEVER_TRACE"] = "1"
        try:
            res = run_bass_kernel_spmd(nc, in_maps, core_ids=list(range(8)),
                                       trace=False)
        finally:
            if prev is None:
                os.environ.pop("BASS_NEVER_TRACE", None)
            else:
                os.environ["BASS_NEVER_TRACE"] = prev
    outs = [np.asarray(res.results[c]["out"], dtype=np.float32)
            for c in range(8)]
    # The last q-block's fo=1 out-projection half is a separate partial.
    for c in range(8):
        outs[c][N - QB:, :] += np.asarray(res.results[c]["out2"],
                                          dtype=np.float32)
    full = np.stack([
        outs[0] + outs[1] + outs[2] + outs[3],
        outs[4] + outs[5] + outs[6] + outs[7],
    ], axis=0)
    kernel.last_exec_time_ns = res.exec_time_ns
    return full



# revision 80
# speedup vs baseline: 1.0000x; 1.0000x over previous
"""Trainium2 Bass kernel for nn_Attention_84739704750279.

Full module: out = (softmax(LN(x) Wq (LN(x) Wk)^T / sqrt(64)) (LN(x) Wv)) Wout

Sharding across 8 NeuronCores: batch (2) x head-groups (4 heads each, 4
groups). Each core computes a partial output [2048, 1024] for its batch using
only its 4 heads; the host sums the 4 partials per batch (row-parallel Wout).

Per-core program (single Bass program, SPMD over 8 cores). All PE operands
are bf16 (x is pre-converted on host; PSUM accumulation stays f32); the
harness error gate is 2e-2 and the bf16 pipeline lands ~6e-3.

Phase A -- LN + transpose + K/V projections (PE-bound, ~50us):
  - x streams in per 256 rows (bf16, halved DMA bytes); weights staggered
    between x chunks in first-use order.
  - bn_stats/bn_aggr on DVE; rstd via Act Sqrt + DVE reciprocal; the
    normalize itself on Pool (otherwise idle), writing bf16 h rows.
  - PE transposes h -> hT via identity matmuls (bf16 = 1 cycle/row), copies
    to SBUF on Act (last chunk on DVE so Act can load the exp table early).
  - V projection per row-tile right after its transposes; K projections of
    chunk cb-1 interleave into chunk cb's row-tile stream (fills DMA/LN
    latency). vaug carries 64 all-ones rows so the context matmul emits the
    softmax denominator replicated across 64 partitions.
  - Q projections and the last chunk's V are NOT done here: they are
    deferred into the phase B backlog, so attention starts ~14us earlier.

Phase B -- attention (Act/exp-bound, ~150us): per (q-block 512, head):
  - scores sT = k_tile^T q in groups of 2 k-tiles into a double-buffered
    PSUM pool; exp fused into the PSUM->SBUF copy on Act (scale=1/8 folded
    in; scores are O(+-8) so no max subtraction needed); ctxT accumulated
    over 16 k-tiles per head.
  - A credit-based backlog scheduler drains deferred PE work (lagged ctx
    groups, head normalizes, out-projection pieces, deferred Q/V) at ~1.5
    matmul-units between exp groups: the Act engine gets a new scores group
    every ~1us and is never starved behind a PE burst. Backlog items are
    tagged with their (q-block, head) pair: force_old() emits everything
    older than the previous head right before a head's tiles are allocated
    (pool ring reuse follows emission order, so a lagged read must never
    outlive its pT/psC/stg ring slot), and the performance trim drain_to()
    fires just after the head's first exp so its burst hides behind Act.
  - Normalize = DVE reciprocal of the replicated denominator rows + one
    tensor_tensor multiply (no partition broadcast).
  - Out-projection is staged per q-block and stored per 256 rows; for the
    last q-block it is split by inner-dim half (fo=0 runs during heads 2-3;
    after the last exp only the fo=1 halves remain -- paired per row-chunk
    into a borrowed scores-ring tile with a single [128, 1024] merge-add --
    and the last head's normalize is emitted per row-chunk slice so the
    first fo=1 piece starts as soon as its slice is ready).

Output partials are stored in bf16 (each core's result is 1 of 4 partial
sums; the host accumulates in f32), halving store traffic.

TimelineSim: 322.5us (baseline) -> 198.3us predicted; measured rel l2 err
6.1e-3 on hardware vs the 2e-2 harness gate.
"""

import numpy as np

NUM_HEAD = 16
HEAD_DIM = 64
DIM = 1024
INNER = NUM_HEAD * HEAD_DIM
B = 2
N = 2048

P = 128
NH = 4            # heads per core
FQK = 2 * NH * HEAD_DIM   # 512 (q block then k block)
FV = NH * HEAD_DIM        # 256
QB = 512          # q-block width for attention
NKT = N // P      # 16 k tiles
DT = DIM // P     # 8 d tiles
GK = 2            # k-tiles per exp group (PSUM double buffered)
SCALE = HEAD_DIM ** -0.5

_CACHE = {}


def _build(apply_affine: bool):
    import concourse.bass as bass
    import concourse.mybir as mybir
    import concourse.tile as tile
    from concourse import bacc
    from concourse.masks import make_identity

    f32 = mybir.dt.float32
    bf16 = mybir.dt.bfloat16
    AF = mybir.ActivationFunctionType
    MUL = mybir.AluOpType.mult

    nc = bacc.Bacc()
    x = nc.declare_dram_parameter("x", [N, DIM], bf16, isOutput=False)
    wqk = nc.declare_dram_parameter("wqk", [DIM, FQK], bf16, isOutput=False)
    wv = nc.declare_dram_parameter("wv", [DIM, FV], bf16, isOutput=False)
    wout = nc.declare_dram_parameter("wout", [FV, DIM], bf16, isOutput=False)
    if apply_affine:
        gamma = nc.declare_dram_parameter("gamma", [DIM], f32, isOutput=False)
        beta = nc.declare_dram_parameter("beta", [DIM], f32, isOutput=False)
    # Output partials in bf16: each core's result is one of 4 partial sums
    # the host accumulates in f32, so the extra rounding is ~1e-3 relative;
    # halves the store bytes and shortens the kernel tail.
    out = nc.declare_dram_parameter("out", [N, DIM], bf16, isOutput=True)
    # The last q-block's fo=1 out-projection half is stored as a separate
    # partial (host adds it): the on-device [128,1024] merge-adds were the
    # kernel's last serialized DVE work, and this lets the fo=0 half store
    # early and the fo=1 copies alternate Act/DVE after the final exp.
    out2 = nc.declare_dram_parameter("out2", [QB, DIM], bf16, isOutput=True)

    with tile.TileContext(nc) as tc:
        with (
            tc.tile_pool(name="const", bufs=1) as const,
            tc.tile_pool(name="persist", bufs=1) as persist,
        ):
            ident = const.tile([P, P], bf16)
            make_identity(nc, ident)
            eps_sb = const.tile([P, 1], f32)
            nc.vector.memset(eps_sb, 1e-5)

            # Weight tiles declared here, DMAs emitted inside phase A after
            # the first x chunk so the x pipeline fills first.
            wqk_sb = const.tile([P, DT, FQK], bf16)
            wv_sb = const.tile([P, DT, FV], bf16)
            wout_sb = const.tile([P, FV // P, DIM], bf16)
            if apply_affine:
                gamma_sb = const.tile([P, DIM], f32)
                nc.sync.dma_start(
                    out=gamma_sb,
                    in_=bass.AP(tensor=gamma.tensor, offset=gamma.offset,
                                ap=[[0, P]] + list(gamma.ap)),
                )
                beta_sb = const.tile([P, DIM], f32)
                nc.sync.dma_start(
                    out=beta_sb,
                    in_=bass.AP(tensor=beta.tensor, offset=beta.offset,
                                ap=[[0, P]] + list(beta.ap)),
                )

            # Persistent activations. vaug rows 64-127 are all-ones: the ctx
            # matmul then yields the softmax denominator replicated across
            # 64 partitions, so no partition-broadcast is needed to
            # normalize.
            qkT = persist.tile([P, FQK // P, N], bf16)      # [128, 4, 2048]
            vaug = persist.tile([P, NKT, NH, 2 * HEAD_DIM], bf16)
            ctxa = persist.tile([P, FV // P, N], bf16)      # [128, 2, 2048]
            nc.gpsimd.memset(vaug[:, :, :, HEAD_DIM:], 1.0)

            # ---------------- Phases A+B, SBUF pools shared ----------------
            with (
                tc.tile_pool(name="xln", bufs=4) as xp,
                tc.tile_pool(name="stat", bufs=10) as sp,
                tc.tile_pool(name="hrow", bufs=6) as hrp,
                tc.tile_pool(name="ht", bufs=4) as hp,
                tc.tile_pool(name="pT", bufs=2) as ppool,
                tc.tile_pool(name="small", bufs=4) as smp,
                tc.tile_pool(name="ostg", bufs=2) as osg,
            ):
                # ---- Phase A: LN, transpose, K projections, v ----
                # Q projections are deferred into the phase B backlog (the
                # q-block qb only needs its Q when its attention runs), which
                # lets the Act-bound attention phase start ~14us earlier.
                hTs = {}
                q_items = {}    # cb -> [(cost, fn), ...] run in phase B

                def make_q_items(cb):
                    items = []
                    holder = {}
                    for fo in (0, 1):
                        for k in range(DT // 2):
                            def itemfn(cb=cb, fo=fo, k=k, holder=holder):
                                if k == 0:
                                    # Shares the out-projection ring ("ps"):
                                    # FIFO order guarantees no other "ps"
                                    # alloc lands mid-accumulation.
                                    holder[fo] = psO.tile([P, QB], f32,
                                                          name="ps",
                                                          uniquify=True)
                                ps = holder[fo]
                                hT = hTs[cb]
                                for dti in (2 * k, 2 * k + 1):
                                    nc.tensor.matmul(
                                        ps,
                                        lhsT=wqk_sb[:, dti,
                                                    fo * P:(fo + 1) * P],
                                        rhs=hT[:, dti, :],
                                        start=(dti == 0),
                                        stop=(dti == DT - 1),
                                    )
                                if k == DT // 2 - 1:
                                    nc.vector.tensor_copy(
                                        out=qkT[:, fo,
                                                cb * QB:(cb + 1) * QB],
                                        in_=ps)
                            items.append((2, itemfn))
                    return items

                with (
                    tc.tile_pool(name="pstr", bufs=2, space="PSUM") as pptr,
                    tc.tile_pool(name="psqk", bufs=2, space="PSUM") as ppqk,
                ):
                    def k_proj(cb, fo, on_act=True):
                        ps = ppqk.tile([P, QB], f32, tag="qk")
                        hT = hTs[cb]
                        for dti in range(DT):
                            nc.tensor.matmul(
                                ps,
                                lhsT=wqk_sb[:, dti, fo * P:(fo + 1) * P],
                                rhs=hT[:, dti, :],
                                start=(dti == 0), stop=(dti == DT - 1),
                            )
                        dst = qkT[:, fo, cb * QB:(cb + 1) * QB]
                        if on_act:
                            nc.scalar.copy(out=dst, in_=ps)
                        else:
                            nc.vector.tensor_copy(out=dst, in_=ps)

                    def v_proj(cb, rsub, hT):
                        ps = ppqk.tile([P, FV], f32, tag="v")
                        for dti in range(DT):
                            nc.tensor.matmul(
                                ps,
                                lhsT=hT[:, dti, rsub * P:(rsub + 1) * P],
                                rhs=wv_sb[:, dti, :],
                                start=(dti == 0), stop=(dti == DT - 1),
                            )
                        nc.scalar.copy(
                            out=vaug[:, cb * 4 + rsub, :, 0:HEAD_DIM],
                            in_=ps.rearrange("p (h d) -> p h d", h=NH),
                        )

                    def make_v_items(cb):
                        # Deferred V projections for the last chunk: run at
                        # the head of the phase B backlog (first needed by
                        # ctx group 6, ~12 exp groups in).
                        items = []
                        for rsub in range(QB // P):
                            holder = {}
                            for piece in range(2):
                                def itemfn(cb=cb, rsub=rsub, piece=piece,
                                           holder=holder):
                                    if piece == 0:
                                        holder[0] = psO.tile(
                                            [P, QB], f32, name="ps",
                                            uniquify=True)
                                    ps = holder[0][:, 0:FV]
                                    hT = hTs[cb]
                                    for dti in range(4 * piece,
                                                     4 * piece + 4):
                                        nc.tensor.matmul(
                                            ps,
                                            lhsT=hT[:, dti,
                                                    rsub * P:(rsub + 1) * P],
                                            rhs=wv_sb[:, dti, :],
                                            start=(dti == 0),
                                            stop=(dti == DT - 1),
                                        )
                                    if piece == 1:
                                        nc.vector.tensor_copy(
                                            out=vaug[:, cb * 4 + rsub, :,
                                                     0:HEAD_DIM],
                                            in_=ps.rearrange(
                                                "p (h d) -> p h d", h=NH),
                                        )
                                items.append((2, itemfn))
                        return items

                    # K projections of chunk cb-1 are interleaved into chunk
                    # cb's row-tile stream to fill PE during DMA/LN latency.
                    pending_k = []
                    for cb in range(N // QB):      # 4 chunks of 512 rows
                        hT = hp.tile([P, DT, QB], bf16)
                        hTs[cb] = hT
                        last = cb == N // QB - 1
                        for half in range(2):      # x DMA per 256 rows
                            r0 = cb * QB + half * (QB // 2)
                            xt = xp.tile([P, 2, DIM], bf16)
                            if cb == 0 and half == 0:
                                # Two half-size loads so the first LN starts
                                # as early as possible.
                                for a2 in range(2):
                                    nc.sync.dma_start(
                                        out=xt[:, a2],
                                        in_=x[r0 + a2 * P:
                                              r0 + (a2 + 1) * P, :])
                            else:
                                nc.sync.dma_start(
                                    out=xt,
                                    in_=x[r0:r0 + QB // 2, :].rearrange(
                                        "(a p) d -> p a d", p=P))
                            # Weight loads staggered between x chunks, in
                            # first-use order (wv -> wqk -> wout), so no x
                            # load waits behind a weight it doesn't need yet.
                            if cb == 0 and half == 0:
                                nc.sync.dma_start(
                                    out=wv_sb,
                                    in_=wv.rearrange("(o p) f -> p o f",
                                                     p=P))
                            elif cb == 0 and half == 1:
                                # K columns (fo 2,3) first: the first K
                                # projection needs them ~1.2us before the Q
                                # half is ever read.
                                nc.sync.dma_start(
                                    out=wqk_sb[:, :, FQK // 2:],
                                    in_=wqk[:, FQK // 2:].rearrange(
                                        "(o p) f -> p o f", p=P))
                                nc.sync.dma_start(
                                    out=wqk_sb[:, :, 0:FQK // 2],
                                    in_=wqk[:, 0:FQK // 2].rearrange(
                                        "(o p) f -> p o f", p=P))
                            elif cb == 1 and half == 0:
                                nc.sync.dma_start(
                                    out=wout_sb,
                                    in_=wout.rearrange("(o p) f -> p o f",
                                                       p=P))
                            for a in range(2):
                                rsub = half * 2 + a
                                xv = xt[:, a].rearrange("p (s f) -> p s f",
                                                        s=2)
                                stats = sp.tile([P, 2, 6], f32)
                                nc.vector.bn_stats(out=stats[:, 0],
                                                   in_=xv[:, 0])
                                nc.vector.bn_stats(out=stats[:, 1],
                                                   in_=xv[:, 1])
                                mv = sp.tile([P, 2], f32)
                                nc.vector.bn_aggr(out=mv, in_=stats)
                                rstd = sp.tile([P, 1], f32)
                                nc.scalar.activation(out=rstd,
                                                     in_=mv[:, 1:2],
                                                     func=AF.Sqrt,
                                                     bias=eps_sb)
                                nc.vector.reciprocal(out=rstd, in_=rstd)
                                hrow = hrp.tile([P, DIM], bf16)
                                if apply_affine:
                                    xf = sp.tile([P, DIM], f32, tag="xf")
                                    nc.vector.tensor_scalar(
                                        out=xf, in0=xt[:, a],
                                        scalar1=mv[:, 0:1], scalar2=rstd,
                                        op0=mybir.AluOpType.subtract,
                                        op1=MUL,
                                    )
                                    nc.vector.tensor_mul(out=xf, in0=xf,
                                                         in1=gamma_sb)
                                    nc.vector.tensor_add(out=xf, in0=xf,
                                                         in1=beta_sb)
                                    nc.vector.tensor_copy(out=hrow, in_=xf)
                                elif cb == 0 and half == 0:
                                    # DVE normalize for the very first
                                    # row-tiles: shorter latency chain than
                                    # Pool while the pipeline fills.
                                    nc.vector.tensor_scalar(
                                        out=hrow, in0=xt[:, a],
                                        scalar1=mv[:, 0:1], scalar2=rstd,
                                        op0=mybir.AluOpType.subtract,
                                        op1=MUL,
                                    )
                                else:
                                    nc.gpsimd.tensor_scalar(
                                        out=hrow, in0=xt[:, a],
                                        scalar1=mv[:, 0:1], scalar2=rstd,
                                        op0=mybir.AluOpType.subtract,
                                        op1=MUL,
                                    )
                                for dhalf in range(2):
                                    tp = pptr.tile([P, 4, P], bf16)
                                    for j in range(4):
                                        dti = dhalf * 4 + j
                                        nc.tensor.transpose(
                                            tp[:, j],
                                            hrow[:, dti * P:(dti + 1) * P],
                                            ident)
                                    dst = hT[:, dhalf * 4:(dhalf + 1) * 4,
                                             rsub * P:(rsub + 1) * P]
                                    # Last chunk's copies on DVE: the Act
                                    # engine winds down early so its exp
                                    # table load overlaps phase A's tail.
                                    if last:
                                        nc.vector.tensor_copy(out=dst,
                                                              in_=tp)
                                    else:
                                        nc.scalar.copy(out=dst, in_=tp)
                                if not last:
                                    # v right away keeps PE busy during the
                                    # next row-tile's DMA/LN latency.
                                    v_proj(cb, rsub, hT)
                                if pending_k:
                                    pending_k.pop(0)()
                        if last:
                            for f in pending_k:
                                f()
                            pending_k = []
                            # K of the last chunk on DVE so the Act engine is
                            # free to switch to the exp table immediately.
                            k_proj(cb, 2, on_act=False)
                            k_proj(cb, 3, on_act=False)
                            q_items[cb] = make_q_items(cb)
                            v_items_last = make_v_items(cb)
                        else:
                            pending_k.extend(
                                (lambda fo=fo, cb=cb: k_proj(cb, fo,
                                                             on_act=False))
                                for fo in (2, 3))
                            if cb == 0:
                                # qb0 needs its Q before attention starts.
                                pending_k.extend(
                                    (lambda fo=fo: k_proj(0, fo,
                                                          on_act=False))
                                    for fo in (0, 1))
                            else:
                                q_items[cb] = make_q_items(cb)

                # ---- Phase B: attention ----
                with (
                    tc.tile_pool(name="pss", bufs=2, space="PSUM") as psS,
                    tc.tile_pool(name="psc", bufs=2, space="PSUM") as psC,
                    tc.tile_pool(name="pso", bufs=2, space="PSUM") as psO,
                ):
                    # Backlog scheduler: PE side-work (lagged ctx groups,
                    # head normalize, out-projection pieces, deferred Q
                    # projections) is queued with a PE-cost weight and
                    # drained at a bounded matmul budget between scores/exp
                    # groups, so the Act engine (the phase B bottleneck) is
                    # fed a new scores group every ~1us and never sits
                    # behind a PE burst.
                    backlog = []   # (pe_matmul_cost, fn)

                    credit = [0.0]

                    def drain(rate):
                        # Credit-carryover drain: averages `rate` matmuls of
                        # backlog per call without bursting PE between two
                        # scores groups (a burst starves the Act engine).
                        credit[0] += rate
                        while backlog and credit[0] >= backlog[0][0]:
                            cost, fn, _ = backlog.pop(0)
                            credit[0] -= cost
                            fn()
                        if not backlog:
                            credit[0] = 0.0

                    def drain_to(maxlen):
                        # Performance trim: bound the standing queue; called
                        # right after a head's first exp so the burst hides
                        # behind Act execution.
                        while len(backlog) > maxlen:
                            cost, fn, _ = backlog.pop(0)
                            fn()

                    def force_old(pair):
                        # Ring-safety: pool ring reuse is by emission order;
                        # a lagged read emitted after its ring buffer wraps
                        # (pT/psC/stg rings are 2 deep) would read clobbered
                        # data. Before pair p's tiles are allocated, every
                        # item from pair <= p-2 must have been emitted.
                        while backlog and backlog[0][2] <= pair - 2:
                            cost, fn, _ = backlog.pop(0)
                            fn()

                    def outproj_do(qb, qc4, do, stg):
                        qc = qb * (QB // P) + qc4
                        ps = psO.tile([P, QB], f32)
                        for fo in range(FV // P):  # 2
                            nc.tensor.matmul(
                                ps,
                                lhsT=ctxa[:, fo, qc * P:(qc + 1) * P],
                                rhs=wout_sb[:, fo,
                                            do * (DIM // 2):
                                            (do + 1) * (DIM // 2)],
                                start=(fo == 0), stop=(fo == FV // P - 1),
                            )
                        dst = stg[:, qc4,
                                  do * (DIM // 2):(do + 1) * (DIM // 2)]
                        nc.vector.tensor_copy(out=dst, in_=ps)
                        if qc4 % 2 == 1 and do == 1:
                            # Store per 256 rows: bounds the kernel tail.
                            r0 = qb * QB + (qc4 - 1) * P
                            nc.sync.dma_start(
                                out=out[r0:r0 + 2 * P, :].rearrange(
                                    "(a p) d -> p a d", p=P),
                                in_=stg[:, qc4 - 1:qc4 + 1, :])

                    backlog.extend((c, f, -1) for c, f in v_items_last)
                    def outproj_fo1_pair(qb, qc4, stg):
                        # Both column halves into one borrowed scores-ring
                        # tile, then a straight PSUM->SBUF copy (alternating
                        # Act/DVE -- Act is idle after the last exp) and a
                        # store to the separate out2 partial; the host adds
                        # it to the fo=0 half.  Runs only in the kernel tail
                        # (scores ring idle; allocations are consecutive in
                        # the final drain, so ring reuse stays
                        # emission-ordered).
                        qc = qb * (QB // P) + qc4
                        # Column halves go to the idle psO and psC rings
                        # (ctx and projections are done by now): four
                        # independent half-slots instead of the 2-deep psS
                        # ring, so the next row-chunk's matmuls are not
                        # gated on a full-width copy.
                        psa = psO.tile([P, QB], f32, name="ps",
                                       uniquify=True)
                        psb = psC.tile([2 * HEAD_DIM, QB], f32, name="cps",
                                       uniquify=True)
                        hd = DIM // 2
                        for do, ps in ((0, psa), (1, psb)):
                            nc.tensor.matmul(
                                ps,
                                lhsT=ctxa[:, 1, qc * P:(qc + 1) * P],
                                rhs=wout_sb[:, 1, do * hd:(do + 1) * hd],
                                start=True, stop=True,
                            )
                        dst = stg[:, qc4, :]
                        if qc4 % 2 == 0:
                            nc.scalar.copy(out=dst[:, 0:hd], in_=psa)
                            nc.vector.tensor_copy(out=dst[:, hd:], in_=psb)
                        else:
                            nc.vector.tensor_copy(out=dst[:, 0:hd], in_=psa)
                            nc.scalar.copy(out=dst[:, hd:], in_=psb)
                        nc.sync.dma_start(
                            out=out2[qc4 * P:(qc4 + 1) * P, :],
                            in_=dst)

                    def outproj_fo_half(qb, qc4, do, stg, fo):
                        qc = qb * (QB // P) + qc4
                        ps = psO.tile([P, QB], f32, name="ps",
                                      uniquify=True)
                        nc.tensor.matmul(
                            ps,
                            lhsT=ctxa[:, fo, qc * P:(qc + 1) * P],
                            rhs=wout_sb[:, fo,
                                        do * (DIM // 2):
                                        (do + 1) * (DIM // 2)],
                            start=True, stop=True,
                        )
                        dst = stg[:, qc4,
                                  do * (DIM // 2):(do + 1) * (DIM // 2)]
                        if fo == 0:
                            nc.vector.tensor_copy(out=dst, in_=ps)
                            if do == 1:
                                # fo=0 half stores early (mid phase B); the
                                # fo=1 half goes to out2 in the tail.
                                r0 = qb * QB + qc4 * P
                                nc.sync.dma_start(
                                    out=out[r0:r0 + P, :],
                                    in_=stg[:, qc4, :])
                        else:
                            nc.vector.tensor_tensor(
                                dst, dst, ps, mybir.AluOpType.add)
                            if do == 1:
                                r0 = qb * QB + qc4 * P
                                nc.sync.dma_start(
                                    out=out[r0:r0 + P, :],
                                    in_=stg[:, qc4, :])

                    for qb in range(N // QB):
                        if qb + 1 in q_items:
                            backlog.extend(
                                (c, f, qb * NH)
                                for c, f in q_items.pop(qb + 1))
                        for h in range(NH):
                            pair = qb * NH + h
                            force_old(pair)
                            p0 = (h % 2) * HEAD_DIM
                            fo_q = h // 2
                            fo_k = FQK // P // 2 + h // 2  # k in fo 2,3
                            qT_h = qkT[p0:p0 + HEAD_DIM, fo_q,
                                       qb * QB:(qb + 1) * QB]
                            kT_h = qkT[p0:p0 + HEAD_DIM, fo_k, :]
                            pT = ppool.tile([P, NKT, QB], bf16)
                            cps = psC.tile([2 * HEAD_DIM, QB], f32)

                            def ctx_group(g, pT=pT, cps=cps, h=h):
                                for j in range(GK):
                                    kt = g * GK + j
                                    nc.tensor.matmul(
                                        cps,
                                        lhsT=vaug[:, kt, h, :],
                                        rhs=pT[:, kt, :],
                                        start=(kt == 0),
                                        stop=(kt == NKT - 1),
                                    )

                            def finish_head(cps=cps, p0=p0, h=h, qb=qb,
                                            sliced=False):
                                recipb = smp.tile([HEAD_DIM, QB], f32,
                                                  tag="recipb")
                                nc.vector.reciprocal(
                                    out=recipb,
                                    in_=cps[HEAD_DIM:2 * HEAD_DIM, :])
                                n0 = qb * QB
                                if sliced:
                                    # Tail: per row-chunk slices so the
                                    # first fo=1 out-proj piece starts as
                                    # soon as its slice is normalized.
                                    for qc4 in range(QB // P):
                                        sl = slice(qc4 * P, (qc4 + 1) * P)
                                        nc.vector.tensor_tensor(
                                            ctxa[p0:p0 + HEAD_DIM, h // 2,
                                                 n0 + qc4 * P:
                                                 n0 + (qc4 + 1) * P],
                                            cps[0:HEAD_DIM, sl],
                                            recipb[:, sl], MUL)
                                else:
                                    dst = ctxa[p0:p0 + HEAD_DIM, h // 2,
                                               n0:n0 + QB]
                                    nc.vector.tensor_tensor(
                                        dst, cps[0:HEAD_DIM, :], recipb,
                                        MUL)

                            for g in range(NKT // GK):
                                ps = psS.tile([P, GK * QB], f32)
                                for j in range(GK):
                                    kt = g * GK + j
                                    nc.tensor.matmul(
                                        ps[:, j * QB:(j + 1) * QB],
                                        lhsT=kT_h[:, kt * P:(kt + 1) * P],
                                        rhs=qT_h,
                                        start=True, stop=True,
                                    )
                                nc.scalar.activation(
                                    out=pT[:, g * GK:(g + 1) * GK, :],
                                    in_=ps,
                                    func=AF.Exp, scale=float(SCALE))
                                if g == 0:
                                    # Backlog cap fires after the head's
                                    # first exp is in flight: the forced
                                    # burst overlaps Act instead of delaying
                                    # the scores Act is waiting for.
                                    drain_to(14)
                                if g >= 1:
                                    backlog.append(
                                        (GK,
                                         lambda g=g, f=ctx_group: f(g - 1),
                                         pair))
                                drain(1.5)
                            backlog.append(
                                (GK, lambda f=ctx_group: f(NKT // GK - 1),
                                 pair))
                            last_pair = (qb == N // QB - 1 and h == NH - 1)
                            backlog.append(
                                (0,
                                 (lambda f=finish_head: f(sliced=True))
                                 if last_pair else finish_head,
                                 pair))
                            if qb == N // QB - 1 and h == 1:
                                # Last q-block: the fo=0 half of the
                                # out-projection only needs heads 0-1, so it
                                # runs during heads 2-3; only the fo=1 half
                                # (+ add + store) remains after the last
                                # exp, shortening the kernel tail.
                                stg3 = osg.tile([P, QB // P, DIM], bf16,
                                                name="stg", uniquify=True)
                                backlog.extend(
                                    (1, lambda qc4=qc4, do=do:
                                     outproj_fo_half(N // QB - 1, qc4, do,
                                                     stg3, 0), pair)
                                    for qc4 in range(QB // P)
                                    for do in range(2))
                            if qb == N // QB - 1 and h == 3:
                                backlog.extend(
                                    (2, lambda qc4=qc4:
                                     outproj_fo1_pair(N // QB - 1, qc4,
                                                      stg3), pair)
                                    for qc4 in range(QB // P))
                        if qb < N // QB - 1:
                            stg = osg.tile([P, QB // P, DIM], bf16)
                            backlog.extend(
                                (2, lambda qb=qb, qc4=qc4, do=do, stg=stg:
                                 outproj_do(qb, qc4, do, stg),
                                 qb * NH + NH - 1)
                                for qc4 in range(QB // P) for do in range(2))
                    drain_to(0)
    nc.finalize()
    return nc


def _get_nc(apply_affine: bool):
    key = ("nc", apply_affine)
    if key not in _CACHE:
        _CACHE[key] = _build(apply_affine)
    return _CACHE[key]


def kernel(x, ln_gamma, ln_beta, w_qkv, w_out, _trace=False):
    from concourse.bass_utils import run_bass_kernel_spmd
    from concourse import mybir

    npbf16 = mybir.dt.np(mybir.dt.bfloat16)

    x = np.ascontiguousarray(np.asarray(x, dtype=np.float32))
    ln_gamma = np.asarray(ln_gamma, dtype=np.float32)
    ln_beta = np.asarray(ln_beta, dtype=np.float32)
    w_qkv = np.asarray(w_qkv, dtype=np.float32)
    w_out = np.asarray(w_out, dtype=np.float32)

    apply_affine = not (np.all(ln_gamma == 1.0) and np.all(ln_beta == 0.0))
    nc = _get_nc(apply_affine)

    wq = w_qkv[:, 0 * INNER:1 * INNER]
    wk = w_qkv[:, 1 * INNER:2 * INNER]
    wv = w_qkv[:, 2 * INNER:3 * INNER]

    in_maps = []
    for c in range(8):
        bi, hg = divmod(c, 4)
        fs = hg * NH * HEAD_DIM
        fe = fs + NH * HEAD_DIM
        m = {
            "x": np.ascontiguousarray(x[bi]).astype(npbf16),
            "wqk": np.ascontiguousarray(
                np.concatenate([wq[:, fs:fe], wk[:, fs:fe]],
                               axis=1)).astype(npbf16),
            "wv": np.ascontiguousarray(wv[:, fs:fe]).astype(npbf16),
            "wout": np.ascontiguousarray(w_out[fs:fe, :]).astype(npbf16),
        }
        if apply_affine:
            m["gamma"] = ln_gamma
            m["beta"] = ln_beta
        in_maps.append(m)

    import os
    try:
        res = run_bass_kernel_spmd(nc, in_maps, core_ids=list(range(8)),
                                   trace=_trace)
    except ModuleNotFoundError:
        # Tracing hooks unavailable in this environment; run untraced
        # (BASS_NEVER_TRACE also overrides a BASS_TRACE env setting).
        prev = os.environ.get("BASS_NEVER_TRACE")
        os.environ["BASS_NEVER_TRACE"] = "1"
        try:
            res = run_bass_kernel_spmd(nc, in_maps, core_ids=list(range(8)),
                                       trace=False)
        finally:
            if prev is None:
                os.environ.pop("BASS_NEVER_TRACE", None)
            else:
                os.environ["BASS_NEVER_TRACE"] = prev
    outs = [np.asarray(res.results[c]["out"], dtype=np.float32)
            for c in range(8)]
    # The last q-block's fo=1 out-projection half is a separate partial.
    for c in range(8):
        outs[c][N - QB:, :] += np.asarray(res.results[c]["out2"],
                                          dtype=np.float32)
    full = np.stack([
        outs[0] + outs[1] + outs[2] + outs[3],
        outs[4] + outs[5] + outs[6] + outs[7],
    ], axis=0)
    kernel.last_exec_time_ns = res.exec_time_ns
    return full

